# revision 5
# baseline (speedup 1.0000x reference)
"""Linear (kernel-based) attention on 8 Trainium2 NeuronCores.

Reference computation (per batch b):
    phi_q = relu(q); phi_k = relu(k)
    kv    = phi_k^T @ v                      # [D, D], reduction over n
    out   = (phi_q @ kv) / (sum_d phi_q + eps)

Sharding: batch (4) x sequence-halves (2) -> 8 shards, one per core.
Each core computes a partial kv state over its half sequence; the two
cores sharing a batch AllReduce the small [D, D] state (+nothing else:
the normalizer is local to each row), then each core computes outputs
for its own rows.

Per-core kernel phases (single NEFF, Tile-scheduled):
  1. stream k,v tiles -> relu(k) -> PE matmuls accumulate kv in PSUM;
     in parallel stream q tiles -> relu -> row-sum (denominator) and
     PE-transpose into a [D, n] SBUF buffer (fp32 has no DMA transpose).
  2. AllReduce kv between the 2 cores of each batch (2-rank groups).
  3. PE matmuls out = phi_q^T.T @ kv, scale by 1/denom, store.
"""

import numpy as np

import concourse.bass as bass
import concourse.mybir as mybir
import concourse.tile as tile
from concourse import bacc, bass_utils
from concourse.masks import make_identity

B, N, D = 4, 16384, 256
NCORES = 8
SEQ_SHARDS = 2
NSH = N // SEQ_SHARDS  # 8192 rows per core
P = 128
NT = NSH // P          # 64 row-tiles per core
DT = 2                 # row-tiles per DMA (double tiles)
NDT = NT // DT         # 32
EPS = 1e-8

_cache = {}


def _build():
    f32 = mybir.dt.float32
    nc = bacc.Bacc(
        "TRN2", target_bir_lowering=False, debug=False, num_devices=NCORES
    )
    q = nc.dram_tensor("q", [NSH, D], f32, kind="ExternalInput")
    k = nc.dram_tensor("k", [NSH, D], f32, kind="ExternalInput")
    v = nc.dram_tensor("v", [NSH, D], f32, kind="ExternalInput")
    o = nc.dram_tensor("o", [NSH, D], f32, kind="ExternalOutput")

    # [NSH, D] viewed as [P, NT, D]: partition p holds row t*P + p.
    q_r = q.ap().rearrange("(t p) d -> p t d", p=P)
    k_r = k.ap().rearrange("(t p) d -> p t d", p=P)
    v_r = v.ap().rearrange("(t p) d -> p t d", p=P)
    o_r = o.ap().rearrange("(t p) d -> p t d", p=P)

    Relu = mybir.ActivationFunctionType.Relu

    with tile.TileContext(nc) as tc:
        with (
            tc.tile_pool(name="const", bufs=1) as constp,
            tc.tile_pool(name="loads", bufs=4) as loads,
            tc.tile_pool(name="qt_store", bufs=1) as qtp,
            tc.tile_pool(name="misc", bufs=1) as misc,
            tc.tile_pool(name="outs", bufs=4) as outp,
            tc.tile_pool(name="psum_kv", bufs=1, space="PSUM") as psum_kv_p,
            tc.tile_pool(name="psum_tr", bufs=4, space="PSUM") as psum_tr,
            tc.tile_pool(name="psum_o", bufs=2, space="PSUM") as psum_o,
            tc.tile_pool(name="dram", bufs=1, space="DRAM") as dram,
        ):
            ident = constp.tile([P, P], f32)
            make_identity(nc, ident[:])

            # phi(q)^T, laid out [P, chunk, n]: element (p, c, n) = phi_q[n, c*P+p]
            qT = qtp.tile([P, 2, NSH], f32)
            denom = misc.tile([P, NT], f32)
            recip = misc.tile([P, NT], f32)
            kv_sb = misc.tile([P, 2, D], f32)
            kv_full = misc.tile([P, 2, D], f32)

            # kv accumulators; kv[c*P+p, e] = sum_n phi_k[n, c*P+p] v[n, e].
            # One PSUM bank per accumulation chain: start=True clears the
            # whole bank, so the two chains must not share one.
            kvps = [
                psum_kv_p.tile([P, D], f32, tag=f"kv{c}", name=f"kvps{c}")
                for c in range(2)
            ]

            # ---- Phase 1: stream k, v (kv state) and q (transpose + denom) ----
            for t2 in range(NDT):
                kt = loads.tile([P, DT, D], f32, tag="kt")
                vt = loads.tile([P, DT, D], f32, tag="vt")
                qt = loads.tile([P, DT, D], f32, tag="qt")
                sl = slice(t2 * DT, (t2 + 1) * DT)
                nc.sync.dma_start(kt[:], k_r[:, sl, :])
                nc.sync.dma_start(vt[:], v_r[:, sl, :])
                nc.sync.dma_start(qt[:], q_r[:, sl, :])

                nc.scalar.activation(kt[:], kt[:], Relu)
                nc.scalar.activation(qt[:], qt[:], Relu)
                nc.vector.tensor_reduce(
                    denom[:, sl], qt[:], axis=mybir.AxisListType.X,
                    op=mybir.AluOpType.add,
                )

                for j in range(DT):
                    t = t2 * DT + j
                    for c in range(2):
                        nc.tensor.matmul(
                            kvps[c][:],
                            lhsT=kt[:, j, c * P:(c + 1) * P],
                            rhs=vt[:, j, :],
                            start=(t == 0),
                            stop=(t == NT - 1),
                        )
                    for c in range(2):
                        ps = psum_tr.tile([P, P], f32, tag="tr")
                        nc.tensor.transpose(
                            ps[:], qt[:, j, c * P:(c + 1) * P], ident[:]
                        )
                        dst = qT[:, c, t * P:(t + 1) * P]
                        # split PSUM->SBUF copies across ACT and DVE
                        if (t + c) % 2 == 0:
                            nc.scalar.copy(dst, ps[:])
                        else:
                            nc.vector.tensor_copy(dst, ps[:])

            # denominator epilogue (overlaps the collective)
            nc.vector.tensor_scalar_add(denom[:], denom[:], EPS)
            nc.vector.reciprocal(recip[:], denom[:])

            # ---- Phase 2: AllReduce kv between sequence-half partners ----
            for c in range(2):
                nc.scalar.copy(kv_sb[:, c, :], kvps[c][:])
            cc_in = dram.tile([P, 2, D], f32)
            cc_out = dram.tile([P, 2, D], f32)
            nc.sync.dma_start(cc_in[:], kv_sb[:])
            nc.gpsimd.collective_compute(
                "AllReduce",
                mybir.AluOpType.add,
                replica_groups=[[0, 1], [2, 3], [4, 5], [6, 7]],
                ins=[cc_in.opt()],
                outs=[cc_out.opt()],
            )
            nc.sync.dma_start(kv_full[:], cc_out[:])

            # ---- Phase 3: out rows = phi_q @ kv, scaled by 1/denom ----
            for t2 in range(NDT):
                ot = outp.tile([P, DT, D], f32, tag="ot")
                for j in range(DT):
                    t = t2 * DT + j
                    pso = psum_o.tile([P, D], f32, tag="pso")
                    for c in range(2):
                        nc.tensor.matmul(
                            pso[:],
                            lhsT=qT[:, c, t * P:(t + 1) * P],
                            rhs=kv_full[:, c, :],
                            start=(c == 0),
                            stop=(c == 1),
                        )
                    nc.vector.tensor_scalar_mul(
                        ot[:, j, :], pso[:], recip[:, t:t + 1]
                    )
                sl = slice(t2 * DT, (t2 + 1) * DT)
                nc.sync.dma_start(o_r[:, sl, :], ot[:])

    nc.compile()
    return nc


def _get_nc():
    if "nc" not in _cache:
        _cache["nc"] = _build()
    return _cache["nc"]


def shard_inputs(q, k, v):
    """core c -> batch c//2, sequence half c%2."""
    in_maps = []
    for c in range(NCORES):
        b, h = divmod(c, SEQ_SHARDS)
        rows = slice(h * NSH, (h + 1) * NSH)
        in_maps.append({
            "q": np.ascontiguousarray(q[b, rows]),
            "k": np.ascontiguousarray(k[b, rows]),
            "v": np.ascontiguousarray(v[b, rows]),
        })
    return in_maps


def unshard_outputs(results):
    out = np.empty((B, N, D), np.float32)
    for c in range(NCORES):
        b, h = divmod(c, SEQ_SHARDS)
        out[b, h * NSH:(h + 1) * NSH] = results[c]["o"]
    return out


def kernel(q, k, v):
    q = np.asarray(q, np.float32)
    k = np.asarray(k, np.float32)
    v = np.asarray(v, np.float32)
    nc = _get_nc()
    res = bass_utils.run_bass_kernel_spmd(
        nc, shard_inputs(q, k, v), core_ids=list(range(NCORES))
    )
    return unshard_outputs(res.results)


# revision 6
# speedup vs baseline: 1.1755x; 1.1755x over previous
"""Linear (kernel-based) attention on 8 Trainium2 NeuronCores.

Reference computation (per batch b):
    phi_q = relu(q); phi_k = relu(k)
    kv    = phi_k^T @ v                      # [D, D], reduction over n
    out   = (phi_q @ kv) / (sum_d phi_q + eps)

Sharding: batch (4) x sequence-halves (2) -> 8 shards, one per core.
Each core computes a partial kv state over its half sequence; the two
cores sharing a batch AllReduce the small [D, D] state (+nothing else:
the normalizer is local to each row), then each core computes outputs
for its own rows.

Per-core kernel phases (single NEFF, Tile-scheduled):
  1. stream k,v tiles -> relu(k) -> PE matmuls accumulate kv in PSUM;
     in parallel stream q tiles -> relu -> row-sum (denominator) and
     PE-transpose into a [D, n] SBUF buffer (fp32 has no DMA transpose).
  2. AllReduce kv between the 2 cores of each batch (2-rank groups).
  3. PE matmuls out = phi_q^T.T @ kv, scale by 1/denom, store.
"""

import numpy as np

import concourse.bass as bass
import concourse.mybir as mybir
import concourse.tile as tile
from concourse import bacc, bass_utils
from concourse.masks import make_identity

B, N, D = 4, 16384, 256
NCORES = 8
SEQ_SHARDS = 2
NSH = N // SEQ_SHARDS  # 8192 rows per core
P = 128
NT = NSH // P          # 64 row-tiles per core
DT = 4                 # row-tiles per DMA batch
NDT = NT // DT         # 32
EPS = 1e-8

_cache = {}


def _build():
    f32 = mybir.dt.float32
    nc = bacc.Bacc(
        "TRN2", target_bir_lowering=False, debug=False, num_devices=NCORES
    )
    q = nc.dram_tensor("q", [NSH, D], f32, kind="ExternalInput")
    k = nc.dram_tensor("k", [NSH, D], f32, kind="ExternalInput")
    v = nc.dram_tensor("v", [NSH, D], f32, kind="ExternalInput")
    o = nc.dram_tensor("o", [NSH, D], f32, kind="ExternalOutput")

    # [NSH, D] viewed as [P, NT, D]: partition p holds row t*P + p.
    q_r = q.ap().rearrange("(t p) d -> p t d", p=P)
    k_r = k.ap().rearrange("(t p) d -> p t d", p=P)
    v_r = v.ap().rearrange("(t p) d -> p t d", p=P)
    o_r = o.ap().rearrange("(t p) d -> p t d", p=P)

    Relu = mybir.ActivationFunctionType.Relu

    with tile.TileContext(nc) as tc:
        with (
            tc.tile_pool(name="const", bufs=1) as constp,
            tc.tile_pool(name="loads", bufs=6) as loads,
            tc.tile_pool(name="qt_store", bufs=1) as qtp,
            tc.tile_pool(name="misc", bufs=1) as misc,
            tc.tile_pool(name="outs", bufs=4) as outp,
            tc.tile_pool(name="psum_kv", bufs=1, space="PSUM") as psum_kv_p,
            tc.tile_pool(name="psum_tr", bufs=4, space="PSUM") as psum_tr,
            tc.tile_pool(name="psum_o", bufs=2, space="PSUM") as psum_o,
            tc.tile_pool(name="dram", bufs=1, space="DRAM") as dram,
        ):
            ident = constp.tile([P, P], f32)
            make_identity(nc, ident[:])

            # phi(q)^T, laid out [P, chunk, n]: element (p, c, n) = phi_q[n, c*P+p]
            qT = qtp.tile([P, 2, NSH], f32)
            denom = misc.tile([P, NT], f32)
            recip = misc.tile([P, NT], f32)
            kv_sb = misc.tile([P, 2, D], f32)
            kv_full = misc.tile([P, 2, D], f32)

            # kv accumulators; kv[c*P+p, e] = sum_n phi_k[n, c*P+p] v[n, e].
            # One PSUM bank per accumulation chain: start=True clears the
            # whole bank, so the two chains must not share one.
            kvps = [
                psum_kv_p.tile([P, D], f32, tag=f"kv{c}", name=f"kvps{c}")
                for c in range(2)
            ]

            # ---- Phase 1: stream k, v (kv state) and q (transpose + denom) ----
            for t2 in range(NDT):
                kt = loads.tile([P, DT, D], f32, tag="kt")
                vt = loads.tile([P, DT, D], f32, tag="vt")
                qt = loads.tile([P, DT, D], f32, tag="qt")
                sl = slice(t2 * DT, (t2 + 1) * DT)
                nc.sync.dma_start(kt[:], k_r[:, sl, :])
                nc.sync.dma_start(vt[:], v_r[:, sl, :])
                nc.sync.dma_start(qt[:], q_r[:, sl, :])

                nc.scalar.activation(kt[:], kt[:], Relu)
                nc.scalar.activation(qt[:], qt[:], Relu)
                nc.vector.tensor_reduce(
                    denom[:, sl], qt[:], axis=mybir.AxisListType.X,
                    op=mybir.AluOpType.add,
                )

                for j in range(DT):
                    t = t2 * DT + j
                    for c in range(2):
                        nc.tensor.matmul(
                            kvps[c][:],
                            lhsT=kt[:, j, c * P:(c + 1) * P],
                            rhs=vt[:, j, :],
                            start=(t == 0),
                            stop=(t == NT - 1),
                        )
                    for c in range(2):
                        ps = psum_tr.tile([P, P], f32, tag="tr")
                        nc.tensor.transpose(
                            ps[:], qt[:, j, c * P:(c + 1) * P], ident[:]
                        )
                        dst = qT[:, c, t * P:(t + 1) * P]
                        # split PSUM->SBUF copies across ACT and DVE
                        if (t + c) % 2 == 0:
                            nc.scalar.copy(dst, ps[:])
                        else:
                            nc.vector.tensor_copy(dst, ps[:])

            # denominator epilogue (overlaps the collective)
            nc.vector.tensor_scalar_add(denom[:], denom[:], EPS)
            nc.vector.reciprocal(recip[:], denom[:])

            # ---- Phase 2: AllReduce kv between sequence-half partners ----
            for c in range(2):
                nc.scalar.copy(kv_sb[:, c, :], kvps[c][:])
            cc_in = dram.tile([P, 2, D], f32)
            cc_out = dram.tile([P, 2, D], f32)
            nc.sync.dma_start(cc_in[:], kv_sb[:])
            nc.gpsimd.collective_compute(
                "AllReduce",
                mybir.AluOpType.add,
                replica_groups=[[0, 1], [2, 3], [4, 5], [6, 7]],
                ins=[cc_in.opt()],
                outs=[cc_out.opt()],
            )
            nc.sync.dma_start(kv_full[:], cc_out[:])

            # ---- Phase 3: out rows = phi_q @ kv, scaled by 1/denom ----
            for t2 in range(NDT):
                ot = outp.tile([P, DT, D], f32, tag="ot")
                for j in range(DT):
                    t = t2 * DT + j
                    pso = psum_o.tile([P, D], f32, tag="pso")
                    for c in range(2):
                        nc.tensor.matmul(
                            pso[:],
                            lhsT=qT[:, c, t * P:(t + 1) * P],
                            rhs=kv_full[:, c, :],
                            start=(c == 0),
                            stop=(c == 1),
                        )
                    nc.vector.tensor_scalar_mul(
                        ot[:, j, :], pso[:], recip[:, t:t + 1]
                    )
                sl = slice(t2 * DT, (t2 + 1) * DT)
                nc.sync.dma_start(o_r[:, sl, :], ot[:])

    nc.compile()
    return nc


def _get_nc():
    if "nc" not in _cache:
        _cache["nc"] = _build()
    return _cache["nc"]


def shard_inputs(q, k, v):
    """core c -> batch c//2, sequence half c%2."""
    in_maps = []
    for c in range(NCORES):
        b, h = divmod(c, SEQ_SHARDS)
        rows = slice(h * NSH, (h + 1) * NSH)
        in_maps.append({
            "q": np.ascontiguousarray(q[b, rows]),
            "k": np.ascontiguousarray(k[b, rows]),
            "v": np.ascontiguousarray(v[b, rows]),
        })
    return in_maps


def unshard_outputs(results):
    out = np.empty((B, N, D), np.float32)
    for c in range(NCORES):
        b, h = divmod(c, SEQ_SHARDS)
        out[b, h * NSH:(h + 1) * NSH] = results[c]["o"]
    return out


def kernel(q, k, v):
    q = np.asarray(q, np.float32)
    k = np.asarray(k, np.float32)
    v = np.asarray(v, np.float32)
    nc = _get_nc()
    res = bass_utils.run_bass_kernel_spmd(
        nc, shard_inputs(q, k, v), core_ids=list(range(NCORES))
    )
    return unshard_outputs(res.results)
